# revision 26
# baseline (speedup 1.0000x reference)
"""Trainium2 Bass kernel for nn_Attention_37056977830181.

Head-sharded (tensor-parallel) multi-head attention over 8 NeuronCores:
each core computes 2 of the 16 heads end-to-end (QKV projection, per-head
RMSNorm, softmax attention, output-projection partial sum); the host sums
the 8 partial projection outputs.

Cross-batch software pipeline: the ACT-bound attention jc-loop of batch
b-1 is issued interleaved with the QKV/V-phase micro-tasks of batch b, so
PE/DVE fill the gaps under the continuous exp stream on ACT.  RMSNorm's
rsqrt runs as exp(-0.5*ln(var)) so the whole kernel uses a single ACT
table set (natural_log_exp_and_others) -- no table switches -- and the
variance path needs no DVE copy or reciprocal.  The projection output is
drained in bf16 (halves DVE copy + HBM write traffic); the host sums the
8 partial outputs in fp32.

Layout (all big matmuls bf16, fp32 accumulation):
  - x pre-transposed on host to xT [B, C, N]; qT/kT produced directly in
    transposed form [128, N] (two heads stacked by 64 partitions).
  - per-head variance via block-diagonal ones matmul (result broadcast
    across each head's 64 partitions), ln on ACT directly from PSUM.
  - v in natural layout [N, 64+1] per head with a ones column so the
    attention matmul's 65th row accumulates the softmax denominator.
  - scoresT [keys, queries] per head, two heads side by side in one
    2-bank PSUM tile; exp on ACT reads PSUM, emits bf16 p-tiles;
    normalization folded in after attn@v via broadcast-DMA'd reciprocal.
"""

import dataclasses
import numpy as np
import ml_dtypes

B, N, C = 4, 2048, 1024
H = 16
D = C // H
SCALE = D**-0.5
EPS = 1e-6
N_CORES = 8
HPC = H // N_CORES  # heads per core = 2
DD = HPC * D  # per-core channel block = 128

bf16 = ml_dtypes.bfloat16

_COMPILED = {}


def _row_bcast(ap, rows):
    """View a [1, F] SBUF AP as [1, rows, F] with a 0-step middle dim, so a
    DMA with a [rows, F] destination replicates the row across partitions."""
    f = ap.shape[-1]
    (pstep, pcount), (estep, ecount) = ap.ap[0], ap.ap[-1]
    assert pcount == 1 and ecount == f
    return dataclasses.replace(ap, ap=[[pstep, 1], [0, rows], [estep, f]])


_WAIT_CAPS = {}
_WAIT_SKIP = {"EventSemaphore", "Call", "ISA", "UnconditionalBranch"}
_WAIT_DEFAULT_CAP = 1
_NOP_CAP = 1


def _split_waits(nc):
    """Walrus's per-instruction-struct sync-wait slots are limited (e.g. the
    self-loading-weights matmul struct takes 1, ACTIVATE takes 2).  Move
    excess waits onto no-op instructions inserted just before, on the same
    engine, preserving execution order semantics."""
    import concourse.mybir as mybir

    nid = [0]
    for f in nc.m.functions:
        for bb in f.blocks:
            out = []
            for inst in bb.instructions:
                si = inst.sync_info
                waits = list(si.on_wait) if si is not None and si.on_wait else []
                cap = (
                    10**9
                    if inst.opcode in _WAIT_SKIP
                    else _WAIT_CAPS.get(inst.opcode, _WAIT_DEFAULT_CAP)
                )
                if len(waits) > cap:
                    excess = waits[: len(waits) - cap]
                    keep = waits[len(waits) - cap :]
                    for j in range(0, len(excess), _NOP_CAP):
                        nop = mybir.InstNoOp(
                            name=f"I-waitsplit-{nid[0]}", ins=[], outs=[]
                        )
                        nid[0] += 1
                        nop.engine = inst.engine
                        nop.bass_nofuse = True
                        nop.sync_info = mybir.SyncInfo(
                            on_wait=excess[j : j + _NOP_CAP], on_update=[]
                        )
                        out.append(nop)
                    inst.sync_info = mybir.SyncInfo(
                        on_wait=keep, on_update=list(si.on_update or [])
                    )
                out.append(inst)
            bb.instructions[:] = out


def build_program(reps=1, hw_loop=0, _strip=()):
    """reps: python-unrolled repetitions.  hw_loop: if >0, wrap the body in a
    Tile For_i hardware loop of that many iterations (for timing runs).
    _strip: dev-only timing bisection {"drain","proj","xtdma","ydma","rb2"}."""
    import contextlib
    import concourse.bass as bass
    import concourse.mybir as mybir
    import concourse.tile as tile

    F32 = mybir.dt.float32
    BF16 = mybir.dt.bfloat16
    AF = mybir.ActivationFunctionType
    AOp = mybir.AluOpType

    nc = bass.Bass(
        "TRN2",
        target_bir_lowering=False,
        debug=False,
        enable_asserts=True,
        num_devices=N_CORES,
    )

    xt_d = nc.dram_tensor("xt", [B, C, N], BF16, kind="ExternalInput").ap()
    wq_d = nc.dram_tensor("wq", [128, 1024], BF16, kind="ExternalInput").ap()
    wk_d = nc.dram_tensor("wk", [128, 1024], BF16, kind="ExternalInput").ap()
    wv_d = nc.dram_tensor("wv", [128, 1024], BF16, kind="ExternalInput").ap()
    pw_d = nc.dram_tensor("pw", [DD, C], BF16, kind="ExternalInput").ap()
    qw_d = nc.dram_tensor("qw", [128, 1], F32, kind="ExternalInput").ap()
    kw_d = nc.dram_tensor("kw", [128, 1], F32, kind="ExternalInput").ap()
    onesrep_d = nc.dram_tensor("onesrep", [128, 128], BF16, kind="ExternalInput").ap()
    y_d = nc.dram_tensor("y", [B, N, C], BF16, kind="ExternalOutput").ap()

    NKC = C // 128  # 8 contraction chunks
    NJC = N // 128  # 16 key chunks
    NIH = 4  # i-quarters of 512
    IW = N // NIH  # 512

    with tile.TileContext(nc) as tc:
        with (
            tc.tile_pool(name="const", bufs=1) as cpool,
            tc.tile_pool(name="xt", bufs=2) as xpool,
            tc.tile_pool(name="qk", bufs=2) as qkpool,
            tc.tile_pool(name="v", bufs=2) as vpool,
            tc.tile_pool(name="work", bufs=2) as wpool,
            tc.tile_pool(name="y", bufs=3) as ypool,
            tc.tile_pool(name="ps", bufs=1, space="PSUM") as ps,
        ):
            # --- constants ---
            w_sb = {}
            for name, dram in (("wq", wq_d), ("wk", wk_d), ("wv", wv_d)):
                t = cpool.tile([128, 1024], BF16, tag=f"c_{name}")
                nc.sync.dma_start(out=t[:], in_=dram)
                w_sb[name] = t
            pw_sb = cpool.tile([DD, C], BF16, tag="c_pw")
            nc.sync.dma_start(out=pw_sb[:], in_=pw_d)
            qw_sb = cpool.tile([128, 1], F32, tag="c_qw")
            nc.sync.dma_start(out=qw_sb[:], in_=qw_d)
            kw_sb = cpool.tile([128, 1], F32, tag="c_kw")
            nc.sync.dma_start(out=kw_sb[:], in_=kw_d)
            onesrep_sb = cpool.tile([128, 128], BF16, tag="c_onesrep")
            nc.sync.dma_start(out=onesrep_sb[:], in_=onesrep_d)
            ones1_sb = cpool.tile([1, 64], F32, tag="c_ones1")
            nc.vector.memset(ones1_sb[:], 1.0)
            eps_sb = cpool.tile([128, 1], F32, tag="c_eps")
            nc.vector.memset(eps_sb[:], EPS)

            loop_ctx = tc.For_i(0, hw_loop, 1) if hw_loop else contextlib.nullcontext()
            with loop_ctx:
              for rep in range(reps):

                # ---- per-batch tile state ----
                def load_xt(b):
                    ts = []
                    for kc in range(NKC):
                        t = xpool.tile([128, N], BF16, tag=f"xt{kc}")
                        if "xtdma" in _strip:
                            nc.vector.memset(t[:], 0.01)
                        else:
                            nc.sync.dma_start(
                                out=t[:], in_=xt_d[b, kc * 128 : (kc + 1) * 128, :]
                            )
                        ts.append(t)
                    return ts

                def make_qv_tasks(b, xt_sb, st):
                    """st: dict getting qnT/knT/v_tiles for batch b.
                    Returns list of closures to issue in order."""
                    qnT = qkpool.tile([128, N], BF16, tag="qnT")
                    knT = qkpool.tile([128, N], BF16, tag="knT")
                    qraw = qkpool.tile([128, N], F32, tag="qraw", bufs=1)
                    kraw = qkpool.tile([128, N], F32, tag="kraw", bufs=1)
                    varb = qkpool.tile([128, 2 * N], F32, tag="varb", bufs=1)
                    v_tiles = [
                        vpool.tile([128, 130], BF16, tag=f"v{jc}", name=f"vt{jc}")
                        for jc in range(NJC)
                    ]
                    st["qnT"], st["knT"], st["v"] = qnT, knT, v_tiles

                    tasks = []

                    def q_mm(wkey, rawT, sl, vsl, half):
                        def fn(pq=[None]):
                            if half == 0:
                                pq_t = ps.tile([128, 512], F32, tag="pq", name="pq")
                                st["pq"] = pq_t
                            pq_t = st["pq"]
                            for kc in range(4 * half, 4 * half + 4):
                                nc.tensor.matmul(
                                    pq_t[:],
                                    w_sb[wkey][:, kc * 128 : (kc + 1) * 128],
                                    xt_sb[kc][:, sl],
                                    start=(kc == 0),
                                    stop=(kc == NKC - 1),
                                )
                            if half == 1:
                                # drain raw values + squares, then per-head
                                # partition sums (block-diag ones matmul) and
                                # ln of the variance straight from PSUM
                                nc.vector.tensor_copy(rawT[:, sl], pq_t[:])
                                sq = wpool.tile([128, 512], BF16, tag="sq", bufs=4)
                                nc.vector.tensor_mul(sq[:], pq_t[:], rawT[:, sl])
                                st["sq"] = sq

                        return fn

                    def q_var(vsl):
                        def fn():
                            psums = ps.tile([128, 512], F32, tag="aux", name="psums")
                            nc.tensor.matmul(
                                psums[:], onesrep_sb[:], st["sq"][:], start=True, stop=True
                            )
                            nc.scalar.activation(
                                varb[:, vsl], psums[:], AF.Ln,
                                bias=eps_sb[:], scale=1.0 / D,
                            )

                        return fn

                    def v_chunk(jc):
                        def fn():
                            pvt = ps.tile([128, 512], F32, tag="aux", name="pv")
                            pv = pvt[:, 0:128]
                            for kc in range(NKC):
                                nc.tensor.matmul(
                                    pv,
                                    xt_sb[kc][:, jc * 128 : (jc + 1) * 128],
                                    w_sb["wv"][:, kc * 128 : (kc + 1) * 128],
                                    start=(kc == 0),
                                    stop=(kc == NKC - 1),
                                )
                            vt = v_tiles[jc]
                            nc.vector.tensor_copy(vt[:, 0:64], pvt[:, 0:64])
                            nc.vector.tensor_copy(vt[:, 65:129], pvt[:, 64:128])
                            nc.vector.memset(vt[:, 64:65], 1.0)
                            nc.vector.memset(vt[:, 129:130], 1.0)

                        return fn

                    # Q chunks interleaved with V chunks so the pipeline-fill
                    # step (no attention to hide under) overlaps PE matmuls
                    # with the Q chain's DVE/ACT hops
                    ci = 0
                    for ti, (wkey, rawT) in enumerate((("wq", qraw), ("wk", kraw))):
                        for ncq in range(N // 512):
                            sl = slice(ncq * 512, (ncq + 1) * 512)
                            vsl = slice(ti * N + ncq * 512, ti * N + (ncq + 1) * 512)
                            tasks.append(q_mm(wkey, rawT, sl, vsl, 0))
                            tasks.append(q_mm(wkey, rawT, sl, vsl, 1))
                            tasks.append(q_var(vsl))
                            tasks.append(v_chunk(2 * ci))
                            tasks.append(v_chunk(2 * ci + 1))
                            ci += 1

                    def rsqrt():
                        # one exp(-0.5*ln v) = v^-0.5 per batch; same ACT
                        # table set as the softmax exps
                        nc.scalar.activation(varb[:], varb[:], AF.Exp, scale=-0.5)

                    tasks.append(rsqrt)

                    def norm(ti, rawT, wcol, dstT, ncq):
                        sl = slice(ncq * 512, (ncq + 1) * 512)
                        vsl = slice(ti * N + ncq * 512, ti * N + (ncq + 1) * 512)

                        def fn():
                            nc.vector.scalar_tensor_tensor(
                                dstT[:, sl],
                                rawT[:, sl],
                                wcol[:],
                                varb[:, vsl],
                                op0=AOp.mult,
                                op1=AOp.mult,
                            )

                        return fn

                    for ncq in range(N // 512):
                        tasks.append(norm(0, qraw, qw_sb, qnT, ncq))
                        tasks.append(norm(1, kraw, kw_sb, knT, ncq))
                    return tasks

                def att_step(b, st, inject, pending):
                    """Issue attention for batch b; call inject() after each jc
                    iteration; drain proj tasks go through `pending`.

                    The attn@v pair for iteration i is issued at iteration i+2,
                    so every cross-engine dependency (scores->exp, exp->attn@v)
                    has at least a full iteration of slack: the only
                    loop-carried serial chains are ACT's exp stream and PE's
                    own instruction stream."""
                    qnT, knT, v_tiles = st["qnT"], st["knT"], st["v"]
                    from collections import deque as _dq

                    av_pending = _dq()

                    def av_task(ih, jc, acc_h, p):
                        def fn():
                            vt = v_tiles[jc]
                            for h in range(HPC):
                                nc.tensor.matmul(
                                    acc_h[h][:],
                                    vt[:, h * 65 : h * 65 + 65],
                                    p[:, h * IW : (h + 1) * IW],
                                    start=(jc == 0),
                                    stop=(jc == NJC - 1),
                                )
                            if jc == NJC - 1:
                                drain1(ih, acc_h)

                        return fn

                    def drain1(ih, acc_h):
                        if "drain" in _strip:
                            return
                        # reciprocal of both heads' denominator rows into one
                        # [2, IW] tile, then broadcast across each head's 64
                        # partitions via a K=2 ones-matmul (a partition-
                        # broadcast DMA from a single-partition source is
                        # ~6us on HW -- the PE does it in 213ns)
                        # copy both heads' RAW denominator rows to SBUF (plain
                        # 1-cycle/elem copies), broadcast them across each
                        # head's 64 partitions via K=1 ones-matmuls, then ONE
                        # reciprocal over the [128, IW] broadcast tile.  The
                        # iterative-divide reciprocal costs 8 cycles/element
                        # regardless of partition count, so halving the
                        # instruction count halves its ~8.5us/drain cost.
                        dens = []
                        for h in range(HPC):
                            den = wpool.tile([1, IW], F32, tag=f"den{h}", bufs=2)
                            nc.vector.tensor_copy(den[:], acc_h[h][64:65, :])
                            dens.append(den)
                        if "rb2" in _strip:
                            return
                        rb2p = ps.tile([128, IW], F32, tag="aux", name="rb2p")
                        for h in range(HPC):
                            nc.tensor.matmul(
                                rb2p[h * 64 : (h + 1) * 64, :],
                                ones1_sb[:],
                                dens[h][:],
                                start=True,
                                stop=True,
                                tile_position=(0, h * 64),
                            )
                        # quick copy releases the shared PSUM bank in ~0.7us;
                        # the 4.3us iterative-divide then runs SBUF->SBUF off
                        # the bank's critical path
                        rb2s = wpool.tile([128, IW], F32, tag="rb2s", bufs=2)
                        nc.vector.tensor_copy(rb2s[:], rb2p[:])
                        rb2 = wpool.tile([128, IW], F32, tag="rb2", bufs=2)
                        nc.vector.reciprocal(rb2[:], rb2s[:])
                        outTn = wpool.tile([128, IW], BF16, tag="outTn", bufs=3)
                        for h in range(HPC):
                            nc.vector.tensor_mul(
                                outTn[h * 64 : (h + 1) * 64, :],
                                acc_h[h][0:64, :],
                                rb2[h * 64 : (h + 1) * 64, :],
                            )
                        if "proj" in _strip:
                            return
                        for mc in range(IW // 128):
                            for oc in range(C // 512):
                                pending.append(proj_task(b, ih, mc, oc, outTn))

                    for ih in range(NIH):
                        isl = slice(ih * IW, (ih + 1) * IW)
                        acc_h = [
                            ps.tile([65, IW], F32, tag=f"acc{h}", name=f"acc_t{h}")
                            for h in range(HPC)
                        ]
                        for jc in range(NJC):
                            scs = ps.tile(
                                [128, 2 * IW], F32, tag="scs", bufs=2, name="scs"
                            )
                            for h in range(HPC):
                                hs = slice(h * 64, (h + 1) * 64)
                                nc.tensor.matmul(
                                    scs[:, h * IW : (h + 1) * IW],
                                    knT[hs, jc * 128 : (jc + 1) * 128],
                                    qnT[hs, isl],
                                    start=True,
                                    stop=True,
                                    tile_position=(h * 64, 0),
                                )
                            p = wpool.tile([128, 2 * IW], BF16, tag="p", bufs=4)
                            nc.scalar.activation(p[:], scs[:], AF.Exp, scale=SCALE)
                            av_pending.append(av_task(ih, jc, acc_h, p))
                            if len(av_pending) > 2:
                                av_pending.popleft()()
                            inject(jc_slot=True)
                    while av_pending:
                        av_pending.popleft()()

                def proj_task(b, ih, mc, oc, outTn):
                    def fn():
                        yp = ps.tile([128, 512], F32, tag="aux", name="yp")
                        nc.tensor.matmul(
                            yp[:],
                            outTn[:, mc * 128 : (mc + 1) * 128],
                            pw_sb[:, oc * 512 : (oc + 1) * 512],
                            start=True,
                            stop=True,
                        )
                        ysb = ypool.tile([128, 512], BF16, tag="ysb")
                        nc.vector.tensor_copy(ysb[:], yp[:])
                        if "ydma" in _strip:
                            return
                        qi0 = ih * IW + mc * 128
                        nc.sync.dma_start(
                            out=y_d[b, qi0 : qi0 + 128, oc * 512 : (oc + 1) * 512],
                            in_=ysb[:],
                        )

                    return fn

                # ---- pipelined schedule over batches ----
                from collections import deque

                pending = deque()  # proj drain tasks
                xt_cur = load_xt(0)
                states = {}
                for step in range(B + 1):
                    b_qv = step if step < B else None
                    b_att = step - 1
                    # prefetch next batch's xT while this step runs
                    xt_next = load_xt(step + 1) if step + 1 < B else None
                    qv_tasks = deque()
                    if b_qv is not None:
                        states[b_qv] = {}
                        qv_tasks = deque(
                            make_qv_tasks(b_qv, xt_cur, states[b_qv])
                        )
                    if b_att < 0:
                        # pipeline fill: no attention yet
                        while qv_tasks:
                            qv_tasks.popleft()()
                    else:
                        njc_total = NIH * NJC
                        nqv = len(qv_tasks)
                        frac = [0.0]

                        def inject(jc_slot=False):
                            if pending:
                                pending.popleft()()
                            frac[0] += nqv / njc_total
                            while frac[0] >= 1.0 and qv_tasks:
                                frac[0] -= 1.0
                                qv_tasks.popleft()()

                        att_step(b_att, states[b_att], inject, pending)
                        while qv_tasks:
                            qv_tasks.popleft()()
                        states.pop(b_att, None)
                    if xt_next is not None:
                        xt_cur = xt_next
                while pending:
                    pending.popleft()()

    _split_waits(nc)
    return nc


def _prepare_inputs(x, qkv_w, q_norm_w, k_norm_w, proj_w):
    """Host-side sharding/layout prep. Returns per-core input maps."""
    xt = np.ascontiguousarray(x.transpose(0, 2, 1)).astype(bf16)  # [B, C, N]
    qw_col = np.tile(q_norm_w, HPC).reshape(128, 1).astype(np.float32)
    kw_col = np.tile(k_norm_w, HPC).reshape(128, 1).astype(np.float32)
    onesrep = np.zeros((128, 128), bf16)
    onesrep[0:64, 0:64] = 1
    onesrep[64:128, 64:128] = 1

    in_maps = []
    for c in range(N_CORES):
        rows = slice(DD * c, DD * (c + 1))

        def pack(w):  # [128 rows, C] -> packed lhsT chunks [128, 1024]
            chunks = [
                np.ascontiguousarray(w[:, kc * 128 : (kc + 1) * 128].T)
                for kc in range(C // 128)
            ]
            return np.concatenate(chunks, axis=1).astype(bf16)

        in_maps.append(
            {
                "xt": xt,
                "wq": pack(qkv_w[0 * C :][rows, :]),
                "wk": pack(qkv_w[1 * C + DD * c : 1 * C + DD * (c + 1), :]),
                "wv": pack(qkv_w[2 * C + DD * c : 2 * C + DD * (c + 1), :]),
                "pw": np.ascontiguousarray(proj_w[:, rows].T).astype(bf16),
                "qw": qw_col,
                "kw": kw_col,
                "onesrep": onesrep,
            }
        )
    return in_maps


def run_on_device(in_maps, reps=1, hw_loop=0):
    from concourse.bass_utils import run_bass_kernel_spmd

    key = (reps, hw_loop)
    if key not in _COMPILED:
        _COMPILED[key] = build_program(reps, hw_loop=hw_loop)
    nc = _COMPILED[key]
    res = run_bass_kernel_spmd(nc, in_maps, list(range(N_CORES)))
    return res


def kernel(x, qkv_w, q_norm_w, k_norm_w, proj_w, proj_b):
    x = np.asarray(x, np.float32)
    qkv_w = np.asarray(qkv_w, np.float32)
    proj_w = np.asarray(proj_w, np.float32)
    in_maps = _prepare_inputs(
        x, qkv_w, np.asarray(q_norm_w, np.float32), np.asarray(k_norm_w, np.float32), proj_w
    )
    res = run_on_device(in_maps, reps=1)
    y = np.zeros((B, N, C), np.float32)
    for c in range(N_CORES):
        y += np.asarray(res.results[c]["y"], np.float32)
    y += np.asarray(proj_b, np.float32)[None, None, :]
    return y


# revision 29
# speedup vs baseline: 1.3048x; 1.3048x over previous
"""Trainium2 Bass kernel for nn_Attention_37056977830181.

Head-sharded (tensor-parallel) multi-head attention over 8 NeuronCores:
each core computes 2 of the 16 heads end-to-end (QKV projection, per-head
RMSNorm, softmax attention, output-projection partial sum); the host sums
the 8 partial projection outputs.

Cross-batch software pipeline: the ACT-bound attention jc-loop of batch
b-1 is issued interleaved with the QKV/V-phase micro-tasks of batch b, so
PE/DVE fill the gaps under the continuous exp stream on ACT.  RMSNorm's
rsqrt runs as exp(-0.5*ln(var)) so the whole kernel uses a single ACT
table set (natural_log_exp_and_others) -- no table switches -- and the
variance path needs no DVE copy or reciprocal.  The projection output is
drained in bf16 (halves DVE copy + HBM write traffic); the host sums the
8 partial outputs in fp32.

Layout (all big matmuls bf16, fp32 accumulation):
  - x pre-transposed on host to xT [B, C, N]; qT/kT produced directly in
    transposed form [128, N] (two heads stacked by 64 partitions).
  - per-head variance via block-diagonal ones matmul (result broadcast
    across each head's 64 partitions), ln on ACT directly from PSUM.
  - v in natural layout [N, 64+1] per head with a ones column so the
    attention matmul's 65th row accumulates the softmax denominator.
  - scoresT [keys, queries] per head, two heads side by side in one
    2-bank PSUM tile; exp on ACT reads PSUM, emits bf16 p-tiles;
    normalization folded in after attn@v via broadcast-DMA'd reciprocal.
"""

import dataclasses
import numpy as np
import ml_dtypes

B, N, C = 4, 2048, 1024
H = 16
D = C // H
SCALE = D**-0.5
EPS = 1e-6
N_CORES = 8
HPC = H // N_CORES  # heads per core = 2
DD = HPC * D  # per-core channel block = 128

bf16 = ml_dtypes.bfloat16

_COMPILED = {}


def _row_bcast(ap, rows):
    """View a [1, F] SBUF AP as [1, rows, F] with a 0-step middle dim, so a
    DMA with a [rows, F] destination replicates the row across partitions."""
    f = ap.shape[-1]
    (pstep, pcount), (estep, ecount) = ap.ap[0], ap.ap[-1]
    assert pcount == 1 and ecount == f
    return dataclasses.replace(ap, ap=[[pstep, 1], [0, rows], [estep, f]])


_WAIT_CAPS = {}
_WAIT_SKIP = {"EventSemaphore", "Call", "ISA", "UnconditionalBranch"}
_WAIT_DEFAULT_CAP = 1
_NOP_CAP = 1


def _split_waits(nc):
    """Walrus's per-instruction-struct sync-wait slots are limited (e.g. the
    self-loading-weights matmul struct takes 1, ACTIVATE takes 2).  Move
    excess waits onto no-op instructions inserted just before, on the same
    engine, preserving execution order semantics."""
    import concourse.mybir as mybir

    nid = [0]
    for f in nc.m.functions:
        for bb in f.blocks:
            out = []
            for inst in bb.instructions:
                si = inst.sync_info
                waits = list(si.on_wait) if si is not None and si.on_wait else []
                cap = (
                    10**9
                    if inst.opcode in _WAIT_SKIP
                    else _WAIT_CAPS.get(inst.opcode, _WAIT_DEFAULT_CAP)
                )
                if len(waits) > cap:
                    excess = waits[: len(waits) - cap]
                    keep = waits[len(waits) - cap :]
                    for j in range(0, len(excess), _NOP_CAP):
                        nop = mybir.InstNoOp(
                            name=f"I-waitsplit-{nid[0]}", ins=[], outs=[]
                        )
                        nid[0] += 1
                        nop.engine = inst.engine
                        nop.bass_nofuse = True
                        nop.sync_info = mybir.SyncInfo(
                            on_wait=excess[j : j + _NOP_CAP], on_update=[]
                        )
                        out.append(nop)
                    inst.sync_info = mybir.SyncInfo(
                        on_wait=keep, on_update=list(si.on_update or [])
                    )
                out.append(inst)
            bb.instructions[:] = out


def build_program(reps=1, hw_loop=0, _strip=()):
    """reps: python-unrolled repetitions.  hw_loop: if >0, wrap the body in a
    Tile For_i hardware loop of that many iterations (for timing runs).
    _strip: dev-only timing bisection {"drain","proj","xtdma","ydma","rb2"}."""
    import contextlib
    import concourse.bass as bass
    import concourse.mybir as mybir
    import concourse.tile as tile

    F32 = mybir.dt.float32
    BF16 = mybir.dt.bfloat16
    AF = mybir.ActivationFunctionType
    AOp = mybir.AluOpType

    nc = bass.Bass(
        "TRN2",
        target_bir_lowering=False,
        debug=False,
        enable_asserts=True,
        num_devices=N_CORES,
    )

    xt_d = nc.dram_tensor("xt", [B, C, N], BF16, kind="ExternalInput").ap()
    wq_d = nc.dram_tensor("wq", [128, 1024], BF16, kind="ExternalInput").ap()
    wk_d = nc.dram_tensor("wk", [128, 1024], BF16, kind="ExternalInput").ap()
    wv_d = nc.dram_tensor("wv", [128, 1024], BF16, kind="ExternalInput").ap()
    pw_d = nc.dram_tensor("pw", [DD, C], BF16, kind="ExternalInput").ap()
    qw_d = nc.dram_tensor("qw", [128, 1], F32, kind="ExternalInput").ap()
    kw_d = nc.dram_tensor("kw", [128, 1], F32, kind="ExternalInput").ap()
    onesrep_d = nc.dram_tensor("onesrep", [128, 128], BF16, kind="ExternalInput").ap()
    y_d = nc.dram_tensor("y", [B, N, C], BF16, kind="ExternalOutput").ap()

    NKC = C // 128  # 8 contraction chunks
    NJC = N // 128  # 16 key chunks
    NIH = 4  # i-quarters of 512
    IW = N // NIH  # 512

    with tile.TileContext(nc) as tc:
        with (
            tc.tile_pool(name="const", bufs=1) as cpool,
            tc.tile_pool(name="xt", bufs=2) as xpool,
            tc.tile_pool(name="qk", bufs=2) as qkpool,
            tc.tile_pool(name="v", bufs=2) as vpool,
            tc.tile_pool(name="work", bufs=2) as wpool,
            tc.tile_pool(name="y", bufs=3) as ypool,
            tc.tile_pool(name="ps", bufs=1, space="PSUM") as ps,
        ):
            # --- constants ---
            w_sb = {}
            for name, dram in (("wq", wq_d), ("wk", wk_d), ("wv", wv_d)):
                t = cpool.tile([128, 1024], BF16, tag=f"c_{name}")
                nc.sync.dma_start(out=t[:], in_=dram)
                w_sb[name] = t
            pw_sb = cpool.tile([DD, C], BF16, tag="c_pw")
            nc.sync.dma_start(out=pw_sb[:], in_=pw_d)
            qw_sb = cpool.tile([128, 1], F32, tag="c_qw")
            nc.sync.dma_start(out=qw_sb[:], in_=qw_d)
            kw_sb = cpool.tile([128, 1], F32, tag="c_kw")
            nc.sync.dma_start(out=kw_sb[:], in_=kw_d)
            onesrep_sb = cpool.tile([128, 128], BF16, tag="c_onesrep")
            nc.sync.dma_start(out=onesrep_sb[:], in_=onesrep_d)
            ones1_sb = cpool.tile([33, 64], F32, tag="c_ones1")
            nc.vector.memset(ones1_sb[:], 1.0)
            eps_sb = cpool.tile([128, 1], F32, tag="c_eps")
            nc.vector.memset(eps_sb[:], EPS)

            loop_ctx = tc.For_i(0, hw_loop, 1) if hw_loop else contextlib.nullcontext()
            with loop_ctx:
              for rep in range(reps):

                # ---- per-batch tile state ----
                def load_xt(b):
                    ts = []
                    for kc in range(NKC):
                        t = xpool.tile([128, N], BF16, tag=f"xt{kc}")
                        if "xtdma" in _strip:
                            nc.vector.memset(t[:], 0.01)
                        else:
                            nc.sync.dma_start(
                                out=t[:], in_=xt_d[b, kc * 128 : (kc + 1) * 128, :]
                            )
                        ts.append(t)
                    return ts

                def make_qv_tasks(b, xt_sb, st):
                    """st: dict getting qnT/knT/v_tiles for batch b.
                    Returns list of closures to issue in order."""
                    qnT = qkpool.tile([128, N], BF16, tag="qnT")
                    knT = qkpool.tile([128, N], BF16, tag="knT")
                    qraw = qkpool.tile([128, N], F32, tag="qraw", bufs=1)
                    kraw = qkpool.tile([128, N], F32, tag="kraw", bufs=1)
                    varb = qkpool.tile([128, 2 * N], F32, tag="varb", bufs=1)
                    v_tiles = [
                        vpool.tile([128, 130], BF16, tag=f"v{jc}", name=f"vt{jc}")
                        for jc in range(NJC)
                    ]
                    st["qnT"], st["knT"], st["v"] = qnT, knT, v_tiles

                    tasks = []

                    def q_mm(wkey, rawT, sl, vsl, half):
                        def fn(pq=[None]):
                            if half == 0:
                                pq_t = ps.tile([128, 512], F32, tag="pq", name="pq")
                                st["pq"] = pq_t
                            pq_t = st["pq"]
                            for kc in range(4 * half, 4 * half + 4):
                                nc.tensor.matmul(
                                    pq_t[:],
                                    w_sb[wkey][:, kc * 128 : (kc + 1) * 128],
                                    xt_sb[kc][:, sl],
                                    start=(kc == 0),
                                    stop=(kc == NKC - 1),
                                )
                            if half == 1:
                                # drain raw values + squares, then per-head
                                # partition sums (block-diag ones matmul) and
                                # ln of the variance straight from PSUM
                                nc.vector.tensor_copy(rawT[:, sl], pq_t[:])
                                sq = wpool.tile([128, 512], BF16, tag="sq", bufs=4)
                                nc.vector.tensor_mul(sq[:], pq_t[:], rawT[:, sl])
                                st["sq"] = sq

                        return fn

                    def q_var(vsl):
                        def fn():
                            psums = ps.tile([128, 512], F32, tag="aux", name="psums")
                            nc.tensor.matmul(
                                psums[:], onesrep_sb[:], st["sq"][:], start=True, stop=True
                            )
                            nc.scalar.activation(
                                varb[:, vsl], psums[:], AF.Ln,
                                bias=eps_sb[:], scale=1.0 / D,
                            )

                        return fn

                    def v_chunk(jc):
                        def fn():
                            pvt = ps.tile([128, 512], F32, tag="aux", name="pv")
                            pv = pvt[:, 0:128]
                            for kc in range(NKC):
                                nc.tensor.matmul(
                                    pv,
                                    xt_sb[kc][:, jc * 128 : (jc + 1) * 128],
                                    w_sb["wv"][:, kc * 128 : (kc + 1) * 128],
                                    start=(kc == 0),
                                    stop=(kc == NKC - 1),
                                )
                            vt = v_tiles[jc]
                            nc.vector.tensor_copy(vt[:, 0:64], pvt[:, 0:64])
                            nc.vector.tensor_copy(vt[:, 65:129], pvt[:, 64:128])
                            nc.vector.memset(vt[:, 64:65], 1.0)
                            nc.vector.memset(vt[:, 129:130], 1.0)

                        return fn

                    # Q chunks interleaved with V chunks so the pipeline-fill
                    # step (no attention to hide under) overlaps PE matmuls
                    # with the Q chain's DVE/ACT hops
                    ci = 0
                    for ti, (wkey, rawT) in enumerate((("wq", qraw), ("wk", kraw))):
                        for ncq in range(N // 512):
                            sl = slice(ncq * 512, (ncq + 1) * 512)
                            vsl = slice(ti * N + ncq * 512, ti * N + (ncq + 1) * 512)
                            tasks.append(q_mm(wkey, rawT, sl, vsl, 0))
                            tasks.append(q_mm(wkey, rawT, sl, vsl, 1))
                            tasks.append(q_var(vsl))
                            tasks.append(v_chunk(2 * ci))
                            tasks.append(v_chunk(2 * ci + 1))
                            ci += 1

                    def rsqrt():
                        # one exp(-0.5*ln v) = v^-0.5 per batch; same ACT
                        # table set as the softmax exps
                        nc.scalar.activation(varb[:], varb[:], AF.Exp, scale=-0.5)

                    tasks.append(rsqrt)

                    def norm(ti, rawT, wcol, dstT, ncq):
                        sl = slice(ncq * 512, (ncq + 1) * 512)
                        vsl = slice(ti * N + ncq * 512, ti * N + (ncq + 1) * 512)

                        def fn():
                            nc.vector.scalar_tensor_tensor(
                                dstT[:, sl],
                                rawT[:, sl],
                                wcol[:],
                                varb[:, vsl],
                                op0=AOp.mult,
                                op1=AOp.mult,
                            )

                        return fn

                    for ncq in range(N // 512):
                        tasks.append(norm(0, qraw, qw_sb, qnT, ncq))
                        tasks.append(norm(1, kraw, kw_sb, knT, ncq))
                    return tasks

                def att_step(b, st, inject, pending):
                    """Issue attention for batch b; call inject() after each jc
                    iteration; drain proj tasks go through `pending`.

                    The attn@v pair for iteration i is issued at iteration i+2,
                    so every cross-engine dependency (scores->exp, exp->attn@v)
                    has at least a full iteration of slack: the only
                    loop-carried serial chains are ACT's exp stream and PE's
                    own instruction stream."""
                    qnT, knT, v_tiles = st["qnT"], st["knT"], st["v"]
                    from collections import deque as _dq

                    av_pending = _dq()

                    def av_task(ih, jc, acc_h, p):
                        def fn():
                            vt = v_tiles[jc]
                            for h in range(HPC):
                                nc.tensor.matmul(
                                    acc_h[h][:],
                                    vt[:, h * 65 : h * 65 + 65],
                                    p[:, h * IW : (h + 1) * IW],
                                    start=(jc == 0),
                                    stop=(jc == NJC - 1),
                                )
                            if jc == NJC - 1:
                                drain1(ih, acc_h)

                        return fn

                    def drain1(ih, acc_h):
                        if "drain" in _strip:
                            return
                        # reciprocal of both heads' denominator rows into one
                        # [2, IW] tile, then broadcast across each head's 64
                        # partitions via a K=2 ones-matmul (a partition-
                        # broadcast DMA from a single-partition source is
                        # ~6us on HW -- the PE does it in 213ns)
                        # copy both heads' RAW denominator rows to SBUF (plain
                        # 1-cycle/elem copies), broadcast them across each
                        # head's 64 partitions via K=1 ones-matmuls, then ONE
                        # reciprocal over the [128, IW] broadcast tile.  The
                        # iterative-divide reciprocal costs 8 cycles/element
                        # regardless of partition count, so halving the
                        # instruction count halves its ~8.5us/drain cost.
                        # Compact reciprocal: put both heads' denominator rows
                        # at partitions 0 / 32 of one tile, 32x32
                        # block-transpose so the 512 dens land on 16 stride-32
                        # columns across 64 partitions, reciprocal just those
                        # (16 elems/lane instead of 512), transpose back.
                        den2 = wpool.tile([64, IW], F32, tag="den2", bufs=2)
                        nc.vector.tensor_copy(den2[0:1, :], acc_h[0][64:65, :])
                        nc.vector.tensor_copy(den2[32:33, :], acc_h[1][64:65, :])
                        if "rb2" in _strip:
                            return
                        dent = wpool.tile([64, IW], F32, tag="dent", bufs=2)
                        nc.vector.transpose(dent[:], den2[:])
                        a = dent[:]
                        (pstep, pcount), (estep, ecount) = a.ap[0], a.ap[-1]
                        strided = dataclasses.replace(
                            a, ap=[[pstep, pcount], [estep * 32, 16]]
                        )
                        nc.vector.reciprocal(strided, strided)
                        nc.vector.transpose(den2[:], dent[:])
                        rb2p = ps.tile([128, IW], F32, tag="aux", name="rb2p")
                        for h in range(HPC):
                            nc.tensor.matmul(
                                rb2p[h * 64 : (h + 1) * 64, :],
                                ones1_sb[32 * h : 32 * h + 1, :],
                                den2[32 * h : 32 * h + 1, :],
                                start=True,
                                stop=True,
                                tile_position=(32 * h, h * 64),
                            )
                        rb2 = wpool.tile([128, IW], F32, tag="rb2", bufs=2)
                        nc.vector.tensor_copy(rb2[:], rb2p[:])
                        outTn = wpool.tile([128, IW], BF16, tag="outTn", bufs=3)
                        for h in range(HPC):
                            nc.vector.tensor_mul(
                                outTn[h * 64 : (h + 1) * 64, :],
                                acc_h[h][0:64, :],
                                rb2[h * 64 : (h + 1) * 64, :],
                            )
                        if "proj" in _strip:
                            return
                        for mc in range(IW // 128):
                            for oc in range(C // 512):
                                pending.append(proj_task(b, ih, mc, oc, outTn))

                    for ih in range(NIH):
                        isl = slice(ih * IW, (ih + 1) * IW)
                        acc_h = [
                            ps.tile([65, IW], F32, tag=f"acc{h}", name=f"acc_t{h}")
                            for h in range(HPC)
                        ]
                        for jc in range(NJC):
                            scs = ps.tile(
                                [128, 2 * IW], F32, tag="scs", bufs=2, name="scs"
                            )
                            for h in range(HPC):
                                hs = slice(h * 64, (h + 1) * 64)
                                nc.tensor.matmul(
                                    scs[:, h * IW : (h + 1) * IW],
                                    knT[hs, jc * 128 : (jc + 1) * 128],
                                    qnT[hs, isl],
                                    start=True,
                                    stop=True,
                                    tile_position=(h * 64, 0),
                                )
                            p = wpool.tile([128, 2 * IW], BF16, tag="p", bufs=4)
                            nc.scalar.activation(p[:], scs[:], AF.Exp, scale=SCALE)
                            av_pending.append(av_task(ih, jc, acc_h, p))
                            if len(av_pending) > 2:
                                av_pending.popleft()()
                            inject(jc_slot=True)
                    while av_pending:
                        av_pending.popleft()()

                def proj_task(b, ih, mc, oc, outTn):
                    def fn():
                        yp = ps.tile([128, 512], F32, tag="aux", name="yp")
                        nc.tensor.matmul(
                            yp[:],
                            outTn[:, mc * 128 : (mc + 1) * 128],
                            pw_sb[:, oc * 512 : (oc + 1) * 512],
                            start=True,
                            stop=True,
                        )
                        ysb = ypool.tile([128, 512], BF16, tag="ysb")
                        nc.vector.tensor_copy(ysb[:], yp[:])
                        if "ydma" in _strip:
                            return
                        qi0 = ih * IW + mc * 128
                        nc.sync.dma_start(
                            out=y_d[b, qi0 : qi0 + 128, oc * 512 : (oc + 1) * 512],
                            in_=ysb[:],
                        )

                    return fn

                # ---- pipelined schedule over batches ----
                from collections import deque

                pending = deque()  # proj drain tasks
                xt_cur = load_xt(0)
                states = {}
                for step in range(B + 1):
                    b_qv = step if step < B else None
                    b_att = step - 1
                    # prefetch next batch's xT while this step runs
                    xt_next = load_xt(step + 1) if step + 1 < B else None
                    qv_tasks = deque()
                    if b_qv is not None:
                        states[b_qv] = {}
                        qv_tasks = deque(
                            make_qv_tasks(b_qv, xt_cur, states[b_qv])
                        )
                    if b_att < 0:
                        # pipeline fill: no attention yet
                        while qv_tasks:
                            qv_tasks.popleft()()
                    else:
                        njc_total = NIH * NJC
                        nqv = len(qv_tasks)
                        frac = [0.0]

                        def inject(jc_slot=False):
                            if pending:
                                pending.popleft()()
                            frac[0] += nqv / njc_total
                            while frac[0] >= 1.0 and qv_tasks:
                                frac[0] -= 1.0
                                qv_tasks.popleft()()

                        att_step(b_att, states[b_att], inject, pending)
                        while qv_tasks:
                            qv_tasks.popleft()()
                        states.pop(b_att, None)
                    if xt_next is not None:
                        xt_cur = xt_next
                while pending:
                    pending.popleft()()

    _split_waits(nc)
    return nc


def _prepare_inputs(x, qkv_w, q_norm_w, k_norm_w, proj_w):
    """Host-side sharding/layout prep. Returns per-core input maps."""
    xt = np.ascontiguousarray(x.transpose(0, 2, 1)).astype(bf16)  # [B, C, N]
    qw_col = np.tile(q_norm_w, HPC).reshape(128, 1).astype(np.float32)
    kw_col = np.tile(k_norm_w, HPC).reshape(128, 1).astype(np.float32)
    onesrep = np.zeros((128, 128), bf16)
    onesrep[0:64, 0:64] = 1
    onesrep[64:128, 64:128] = 1

    in_maps = []
    for c in range(N_CORES):
        rows = slice(DD * c, DD * (c + 1))

        def pack(w):  # [128 rows, C] -> packed lhsT chunks [128, 1024]
            chunks = [
                np.ascontiguousarray(w[:, kc * 128 : (kc + 1) * 128].T)
                for kc in range(C // 128)
            ]
            return np.concatenate(chunks, axis=1).astype(bf16)

        in_maps.append(
            {
                "xt": xt,
                "wq": pack(qkv_w[0 * C :][rows, :]),
                "wk": pack(qkv_w[1 * C + DD * c : 1 * C + DD * (c + 1), :]),
                "wv": pack(qkv_w[2 * C + DD * c : 2 * C + DD * (c + 1), :]),
                "pw": np.ascontiguousarray(proj_w[:, rows].T).astype(bf16),
                "qw": qw_col,
                "kw": kw_col,
                "onesrep": onesrep,
            }
        )
    return in_maps


def run_on_device(in_maps, reps=1, hw_loop=0):
    from concourse.bass_utils import run_bass_kernel_spmd

    key = (reps, hw_loop)
    if key not in _COMPILED:
        _COMPILED[key] = build_program(reps, hw_loop=hw_loop)
    nc = _COMPILED[key]
    res = run_bass_kernel_spmd(nc, in_maps, list(range(N_CORES)))
    return res


def kernel(x, qkv_w, q_norm_w, k_norm_w, proj_w, proj_b):
    x = np.asarray(x, np.float32)
    qkv_w = np.asarray(qkv_w, np.float32)
    proj_w = np.asarray(proj_w, np.float32)
    in_maps = _prepare_inputs(
        x, qkv_w, np.asarray(q_norm_w, np.float32), np.asarray(k_norm_w, np.float32), proj_w
    )
    res = run_on_device(in_maps, reps=1)
    y = np.zeros((B, N, C), np.float32)
    for c in range(N_CORES):
        y += np.asarray(res.results[c]["y"], np.float32)
    y += np.asarray(proj_b, np.float32)[None, None, :]
    return y
